# revision 2
# baseline (speedup 1.0000x reference)
"""Entmax-bisect (alpha-entmax, 10-step bisection) on Trainium2 — band-compaction design.

Math (per row, alpha=1.5 => am1=0.5, hat domain tau_hat = tau/am1):
    mx = max(x); c = mx - 1/am1; dm0 = (1 - (1/d)^am1)/am1
    f(t) >= 0  <=>  Q(t) = sum((x - t)^+^2) >= 1/am1^2
    10 bisection steps on t in [c, c+dm0]; out = p / sum(p), p = ((x-t)^+)^2

Only elements with x >= c can ever contribute (4-25% for randn rows), so each
row's active set is compacted once into a narrow "band" and the 10 bisection
sums run on the band only:

  per tile [128, 4096] fp16:
    DVE  tree-max (tensor_max halvings, fp16 4x) -> mx
    DVE  custom scan op: idx = g*cumsum(g)-1, g = (x>=c)   [int16 ranks]
    GPS  local_scatter: band[rank] = x  (per-partition compaction)
    DVE/ACT 10x: Q = sum((band - tau)^+^2); tau +-= dm0*2^-k   [width w_t]
    ACT  r = relu(s*x - s*tau_fin), s = sqrt(1/Q)
    DVE  out = r*r  (fp16 4x)

Host side: fp16 convert; rows sorted by active-count and dealt round-robin to
the 8 cores so all cores share one width profile (w_t = per-tile max count,
64-aligned); inverse permutation + f32 upconvert on the way out. Compile is
per width-profile (cached).
"""

import math
from operator import add as _op_add

import numpy as np

import concourse.bass as bass  # noqa: F401
import concourse.tile as tile
from concourse import bacc, mybir
from concourse.bass_utils import run_bass_kernel_spmd

N_CORES = 8
D = 4096
N_ITER = 10
P = 128

# steps 1..DVE_STEPS run on DVE, the rest on ACT (contiguous blocks per tile
# kill the cross-engine ping-pong: in-order engines head-block per crossing)
DVE_STEPS = 4
STRIDE = 2
# bubble stages between scatter and the first bisection step (lets the DVE
# work on other tiles instead of head-of-line blocking on the scatter sem)
BUBBLES = 3

TRACE = False
LAST_RESULT = None

_NC_CACHE = {}


# ---------- runtime registration of custom DVE ops ----------------------

def _register_dve_op(op_name, spec):
    from concourse import dve_ops as DO
    from concourse.dve_spec import lower, _has_src1 as has_src1
    from concourse.dve_uop import DveOpSpec

    for o in DO.OPS:
        if o.name == op_name:
            return o
    row = DO._CUSTOM_DVE_ROW_BASE + len(DO.OPS)
    assert row < 0x20
    shas = {}
    for ver in ("v3", "v4"):
        s = DveOpSpec(name=op_name, opcode=row, uops=lower(spec, ver=ver),
                      rd1_en=has_src1(spec))
        shas[ver] = s.sha(ver)
    op = DO.DveOp(op_name, spec, subdim=False, uops_sha=shas)
    DO.OPS.append(op)
    DO._SUB_OPCODE_FOR_NAME[op_name] = row
    DO.CUSTOM_DVE_SPECS[op_name] = spec
    return op


def _get_ops():
    from concourse.dve_spec import (
        Spec, Src0, Src1, C0, C1, C2, Zero, One, relu, select, sq, scan, AluOp,
    )

    # idx = g*cumsum(g) - 1 ; g = (x - c >= 0)   -> rank-1 or -1
    def _scan_ref(in0, in1, s0, s1, imm2):
        g = (in0.astype(np.float32) - s0 >= 0).astype(np.float32)
        return (np.cumsum(g, axis=1) * g - 1.0).astype(np.float32)

    g = select(Src0 - C0 >= Zero, One, Zero)
    SCANIDX = _register_dve_op(
        "ENTMAX_SCANIDX_ANT",
        Spec(body=g * scan(AluOp.ADD, g) - One, reference=_scan_ref),
    )

    # nu = -tau chain.  out = relu(x + nu)^2 ; accum = init + sum(out)
    def _step_ref(in0, in1, c0, c1, c2):
        b = np.maximum(in0.astype(np.float32) + c0, 0.0) ** 2
        b = b.astype(np.float32)
        return b, c1 + b.reshape(b.shape[0], -1).sum(axis=-1, keepdims=True)

    STEPB = _register_dve_op(
        "ENTMAX_STEPBN_ANT",
        Spec(body=sq(relu(Src0 + C0)), accum=_op_add, accum_init=C1,
             reference=_step_ref),
    )

    # nu' = nu - (Q + C2 >= 0 ? C0 : -C0)   [C2 = -1/am1^2, C0 = g_{k+1}]
    UPD = _register_dve_op(
        "ENTMAX_NUPM_ANT",
        Spec(body=Src1 + select(Src0 + C2 >= Zero, Zero - C0, C0),
             reference=lambda in0, in1, s0, s1, imm2: (
                 in1 - np.where(in0 + imm2 >= 0, s0, -s0)).astype(np.float32)),
    )
    return SCANIDX, STEPB, UPD


def _build(am1: float, widths):
    """Single-core program: 16 tiles of [128, D]; widths[t] = band width."""
    f32 = mybir.dt.float32
    f16 = mybir.dt.float16
    i16 = mybir.dt.int16
    AF = mybir.ActivationFunctionType
    OP = mybir.AluOpType
    AX = mybir.AxisListType
    SCANIDX, STEPB, UPD = _get_ops()

    ntiles = len(widths)
    rows = ntiles * P
    c_lo = 1.0 / am1                                   # c = mx - c_lo
    pw = float(np.power(np.float32(1.0 / D), np.float32(am1)))
    dm0 = float((np.float32(1.0) - np.float32(pw)) / np.float32(am1))
    negK = -1.0 / (am1 * am1)                          # Q threshold (negated)

    nc = bacc.Bacc(None, target_bir_lowering=False)
    Xd = nc.declare_dram_parameter("X", [rows, D], f16, isOutput=False)
    Od = nc.declare_dram_parameter("OUT", [rows, D], f16, isOutput=True)

    HC2 = D // 2

    with tile.TileContext(nc) as tc:
        with (
            tc.tile_pool(name="xp", bufs=8) as xp,
            tc.tile_pool(name="ip", bufs=2) as ip,
            tc.tile_pool(name="bp", bufs=6) as bp,
            tc.tile_pool(name="dp", bufs=3) as dp,
            tc.tile_pool(name="rp", bufs=2) as rp,
            tc.tile_pool(name="st", bufs=32) as st,
            tc.tile_pool(name="kp", bufs=1) as kp,
        ):
            negk_t = kp.tile([P, 1], f32, tag="negk", name="negk")
            nc.vector.memset(negk_t[:], negK)
            xt, idx, band, rb, tau, qq, ct = {}, {}, {}, {}, {}, {}, {}

            def emit_dma(t):
                xt[t] = xp.tile([P, D], f16, tag="xt", name="xt")
                nc.sync.dma_start(out=xt[t][:, :HC2],
                                  in_=Xd[t * P:(t + 1) * P, :HC2])
                nc.sync.dma_start(out=xt[t][:, HC2:],
                                  in_=Xd[t * P:(t + 1) * P, HC2:])

            def emit_max(t):
                # 1-port reduce_max (2-port tensor_tensor ops starve while
                # the gpsimd scatter is hammering SBUF)
                mx = st.tile([P, 1], f32, tag="mx", name="mx")
                nc.vector.reduce_max(mx[:], xt[t][:], axis=AX.X)
                # c = mx - 1/am1 ; nu_1 = -(c + dm0/2) = (c_lo - dm0/2) - mx
                ct[t] = st.tile([P, 1], f32, tag="ct", name="ct")
                nc.vector.tensor_scalar(ct[t][:], mx[:], c_lo, None,
                                        OP.subtract)
                tau[t] = st.tile([P, 1], f32, tag="tau", name="nu")
                nc.vector.tensor_scalar(tau[t][:], mx[:],
                                        c_lo - dm0 * 0.5, -1.0,
                                        OP.subtract, OP.mult)

            def emit_scan(t):
                idx[t] = ip.tile([P, D], i16, tag="idx", name="idx")
                nc.vector._custom_dve(SCANIDX, out=idx[t][:], in0=xt[t][:],
                                      s0=ct[t][:], s1=0.0, imm2=0.0)

            def emit_scatter(t):
                band[t] = bp.tile([P, widths[t]], f16, tag=f"band{widths[t]}",
                                  name="band")
                nc.gpsimd.local_scatter(band[t][:], xt[t][:], idx[t][:],
                                        channels=P, num_elems=widths[t],
                                        num_idxs=D)
                rb[t] = None

            def emit_step(t, k):
                # Q = sum(relu(band + nu)^2); last step's Q also normalizes
                q = st.tile([P, 1], f32, tag="q", name="q")
                g_next = dm0 * (0.5 ** (k + 1))
                if k > DVE_STEPS:
                    # ACT block: relu -> square+accum -> sign -> nu update,
                    # all on ACT (no DVE round-trip)
                    rbt = dp.tile([P, widths[t]], f16,
                                  tag=f"rb{widths[t]}", name="rbt")
                    nc.scalar.activation(rbt[:], band[t][:], AF.Relu,
                                         bias=tau[t][:], scale=1.0)
                    nc.scalar.activation(rbt[:], rbt[:], AF.Square,
                                         bias=0.0, scale=1.0, accum_out=q[:])
                    if k < N_ITER:
                        # sgn = sign(Q - K); nu' = nu - g*sgn
                        sg = st.tile([P, 1], f32, tag="sg", name="sg")
                        nc.scalar.activation(sg[:], q[:], AF.Sign,
                                             bias=negk_t[:], scale=1.0)
                        tau_new = st.tile([P, 1], f32, tag="tau", name="nu")
                        nc.scalar.activation(tau_new[:], sg[:], AF.Identity,
                                             bias=tau[t][:], scale=-g_next)
                        tau[t] = tau_new
                    else:
                        qq[t] = q
                else:
                    dummy = dp.tile([P, widths[t]], f16,
                                    tag=f"rb{widths[t]}", name="dummy")
                    nc.vector._custom_dve(STEPB, out=dummy[:], in0=band[t][:],
                                          s0=tau[t][:], s1=0.0,
                                          accum_out=q[:])
                    tau_new = st.tile([P, 1], f32, tag="tau", name="nu")
                    nc.vector._custom_dve(UPD, out=tau_new[:], in0=q[:],
                                          in1=tau[t][:], s0=g_next, s1=0.0,
                                          imm2=negK)
                    tau[t] = tau_new

            def emit_final(t):
                # r = relu(x + nu); out = (r*r) * (1/Q)
                rec = st.tile([P, 1], f32, tag="rec", name="rec")
                nc.vector.reciprocal(rec[:], qq[t][:])
                rt = rp.tile([P, D], f16, tag="rt", name="rt")
                nc.scalar.activation(rt[:], xt[t][:], AF.Relu, bias=tau[t][:],
                                     scale=1.0)
                rt2 = rp.tile([P, D], f16, tag="rt2", name="rt2")
                nc.vector.tensor_mul(rt2[:], rt[:], rt[:])
                rt3 = rp.tile([P, D], f16, tag="rt3", name="rt3")
                nc.vector.tensor_scalar(rt3[:], rt2[:], rec[:], None, OP.mult)
                nc.sync.dma_start(out=Od[t * P:(t + 1) * P, :], in_=rt3[:])

            # software pipeline: stage s, tile t runs phase k = s - off[t]
            # phases: 0 dma, 1 max, 2 scan, 3 scatter, bubbles,
            #         4+B..13+B steps, 14+B final
            S0 = 4 + BUBBLES
            offs = [STRIDE * t for t in range(ntiles)]
            last = (offs[-1] if offs else 0) + S0 + N_ITER + 1
            for s in range(last + 1):
                for t in range(ntiles):
                    k = s - offs[t]
                    if k == 0:
                        emit_dma(t)
                    elif k == 1:
                        emit_max(t)
                    elif k == 2:
                        emit_scan(t)
                    elif k == 3:
                        emit_scatter(t)
                    elif S0 <= k < S0 + N_ITER:
                        emit_step(t, k - S0 + 1)
                    elif k == S0 + N_ITER:
                        emit_final(t)

    nc.finalize()
    return nc


def _get_nc(am1: float, widths):
    key = (am1, tuple(widths), DVE_STEPS, BUBBLES, STRIDE)
    if key not in _NC_CACHE:
        _NC_CACHE[key] = _build(am1, widths)
    return _NC_CACHE[key]


def _ensure_ntff_hook():
    """Register the NTFF profile hook that bass_utils needs for trace=True
    under axon; neuter the S3 artifact upload."""
    import sys as _sys
    import types

    import antenv
    import concourse.bass_utils as _bu

    _bu.upload_artifacts = lambda tmpdir: str(tmpdir)
    try:
        from antenv import axon_hooks  # noqa: F401
        return
    except ImportError:
        pass
    from trn_agent_boot.trn_boot import _ntff_profile_via_ctypes

    hook = _ntff_profile_via_ctypes("/opt/axon/libaxon_pjrt.so")
    mod = types.ModuleType("antenv.axon_hooks")
    mod._hook = hook
    mod.get_axon_ntff_profile_hook = lambda: mod._hook

    def _set(h):
        mod._hook = h

    mod.set_axon_ntff_profile_hook = _set
    _sys.modules["antenv.axon_hooks"] = mod
    antenv.axon_hooks = mod


def kernel(X, alpha):
    global LAST_RESULT
    a = float(np.asarray(alpha, dtype=np.float32).reshape(()))
    am1 = a - 1.0
    assert am1 > 0 and math.log2(am1) == round(math.log2(am1)), (
        f"unsupported alpha={a}"
    )

    orig_shape = X.shape
    x16 = np.asarray(X, dtype=np.float32).reshape(-1, D).astype(np.float16)
    rows_total = x16.shape[0]
    assert rows_total % (N_CORES * P) == 0
    rows = rows_total // N_CORES
    ntiles = rows // P

    # per-row active counts (same predicate the device uses, in f32 on the
    # fp16-quantized values) -> sort rows, deal round-robin, width profile
    xf = x16.astype(np.float32)
    mx = xf.max(axis=1, keepdims=True)
    counts = np.count_nonzero(xf >= mx - np.float32(1.0 / am1), axis=1)
    del xf
    perm = np.argsort(-counts, kind="stable")
    csort = counts[perm]
    widths = []
    gper = rows_total // ntiles
    for t in range(ntiles):
        w = int(csort[t * gper:(t + 1) * gper].max())
        widths.append(max(64, ((w + 63) // 64) * 64))

    nc = _get_nc(am1, widths)
    in_maps = [{"X": np.ascontiguousarray(x16[perm[c::N_CORES]])}
               for c in range(N_CORES)]
    if TRACE:
        _ensure_ntff_hook()
    res = None
    for attempt in range(3):
        try:
            res = run_bass_kernel_spmd(nc, in_maps, list(range(N_CORES)),
                                       trace=TRACE)
            break
        except Exception:
            if attempt == 2:
                raise
            import time
            time.sleep(5.0)
    LAST_RESULT = res
    out16 = np.empty((rows_total, D), dtype=np.float16)
    for c in range(N_CORES):
        out16[perm[c::N_CORES]] = res.results[c]["OUT"]
    return np.ascontiguousarray(out16.reshape(orig_shape).astype(np.float32))
